# revision 18
# baseline (speedup 1.0000x reference)
"""Trainium2 Bass kernel for nn_AttentionLayer (B=32, T=2048, D=512).

Computation (per batch b):
    s1 = x0 @ W_a                       # (D,)
    s2 = x1[b] @ W_h                    # (T, D)
    s  = tanh(s1 + s2)                  # (T, D)
    o  = V_a @ s                        # (D,)   (contract T)
    alpha = softmax(o)                  # over D
    out[b] = alpha * sum(x1[b])

Sharding: data-parallel over batch across 8 NeuronCores (4 batches/core),
weights replicated, no cross-core communication.

v2 design notes (HW-measured 105 us vs 128 us v1 baseline; fro rel err
1.18e-2, gate 2e-2):
  v1 was elementwise-bound: ACT 76% / DVE 57% busy on [128,512] fp32 ops
  (~830 ns each incl. fixed overhead) plus per-op accumulator reads
  (ACT 378 ns). PE only 67%, first 35 us at HAM half-clock.
  v2 restructures around per-op fixed costs and engine balance:
  - t processed in 1024-wide halves: tanh/STT ops are [128,1024]
    (fixed cost amortized 2x), transpose PSUM->SBUF copies are [128,1024]
  - tanh keeps the free per-partition bias (s1T) and free scale knob
  - main matmul in fp8e4m3 with perf_mode=DoubleRow (2 k-tiles per
    instruction, ~1.8x bf16 rate, HW-verified 259 ns/MM at N=512);
    W_h pre-scaled by 64 into fp8's normal range, compensated exactly
    by tanh's scale=1/64. accum_out on the cast-copies sums the
    pre-cast fp32 PSUM values, so sum(x1) stays bf16-accurate.
  - x1 transposes as regular bf16 matmuls vs identity pipeline at
    ~67 ns/128x128 chunk (LDWEIGHTS overlaps in the 64-deep window)
  - V_a-weighted t-sums: DVE scalar_tensor_tensor, bf16 inputs
    (fp32 out+accum; the all-bf16 variant wedges TRN2, and the
    STT uop is 1x-only so bf16 only saves SBUF, not cycles)
  - software pipeline: xt production for half u emitted interleaved
    with consumption of half u-1 (skew=1; skew=2 measured worse)
  - copies: 1 on ACT / 3 on DVE per half (ACT queue stalls on the
    PE-critical path if it carries more; 2.5/1.5 measured worse)
  - DMA: batch 0 loaded as 2x2MB SWDGE cast-DMAs (prologue latency),
    batches 1-3 as 4MB each (SWDGE efficiency); W_h via HWDGE fp32 +
    on-chip cast so the first DoubleRow matmul isn't gated on the
    SWDGE queue; 28 dummy matmuls at t=0 warm the HAM clock gate
  - PSUM: 2x tp tiles (2 banks) + 2x mm tiles (2 banks) = 8 banks
  Engine occupancy at 105 us: DVE 64%, PE 59%, ACT 43%, SWDGE DMA 45%.
  Remaining gap to the ~60 us roofline (DMA 47 us/core, PE 52 us) is
  cross-engine dependency stalls + SBUF port contention (+20% per-op
  when fully overlapped) + HAM cold windows; deeper skew and other
  copy/engine splits measured worse (110-123 us).
"""

import numpy as np

B, T, D = 32, 2048, 512
NCORES = 8
BL = B // NCORES          # batches per core
P = 128
KC = D // P               # k chunks (4)
NCH = D // P              # output-d chunks (4)
TH = 512                  # t elements per 512-block
TB = T // TH              # 512-blocks (4)
NH = 2                    # 1024-halves per batch
SUB = TH // P             # 128-subtiles per 512-block (4)
WH_SCALE = 64.0           # fp8 pre-scale for W_h (into normal range)


def build_nc_v2(
    mode: str = "fp8",
    stt_gps: int = 0,
    copies_act: int = 2,
    stt_bf16: bool = False,
    skew: int = 1,
):
    """mode in {"fp8", "bf16"}.

    stt_gps: how many of the 4 per-half STT ops go to GPSIMD (rest DVE).
    NOTE: walrus rejects TensorScalarPtr on Pool — keep 0 on TRN2.
    copies_act: how many of the 4 per-half transpose-copies go to ACT
    (rest DVE).
    stt_bf16: tanh output + V_a in bf16 so the STT reads packed 16-bit
    at 2x rate (out/accum stay fp32; the known TRN2 wedge was the
    all-bf16 variant).
    """
    import concourse.bass as bass
    import concourse.tile as tile
    from concourse import bacc, mybir
    from concourse.masks import make_identity

    f32 = mybir.dt.float32
    bf16 = mybir.dt.bfloat16
    fp8 = mybir.dt.float8e4
    dt_mm = fp8 if mode == "fp8" else bf16
    inv_scale = 1.0 / WH_SCALE if mode == "fp8" else 1.0

    nc = bacc.Bacc("TRN2", target_bir_lowering=False)

    x0s = nc.dram_tensor("x0s", [BL, D], f32, kind="ExternalInput").ap()
    x1s = nc.dram_tensor("x1s", [BL, T, D], f32, kind="ExternalInput").ap()
    wa = nc.dram_tensor("W_a", [D, D], f32, kind="ExternalInput").ap()
    wh = nc.dram_tensor("W_h", [D, D], f32, kind="ExternalInput").ap()
    va = nc.dram_tensor("V_a", [1, T], f32, kind="ExternalInput").ap()
    out = nc.dram_tensor("out", [BL, D], f32, kind="ExternalOutput").ap()

    with tile.TileContext(nc) as tc:
        with (
            tc.tile_pool(name="consts", bufs=1) as consts,
            tc.tile_pool(name="nat", bufs=3) as nat_pool,
            tc.tile_pool(name="xt", bufs=4) as xt_pool,
            tc.tile_pool(name="s", bufs=3) as s_pool,
            tc.tile_pool(name="scr", bufs=2) as scr_pool,
            tc.tile_pool(name="small", bufs=1) as small,
            tc.tile_pool(name="tp_ps", bufs=2, space="PSUM") as tp_pool,
            tc.tile_pool(name="mm_ps", bufs=2, space="PSUM") as mm_pool,
        ):
            units = [(b, h) for b in range(BL) for h in range(NH)]
            nat_tiles = {}

            def load_nat(u):
                # b0 loads in 2 MB halves (prologue latency), b1.. as one
                # 4 MB transfer per batch (better SWDGE efficiency)
                if u >= len(units) or u in nat_tiles:
                    return
                b, h = units[u]
                if b == 0:
                    nt = nat_pool.tile([P, NH * SUB, D], bf16, tag="nath", name="nt")
                    src = x1s[b, h * 1024 : (h + 1) * 1024, :].rearrange(
                        "(u p) d -> p u d", p=P
                    )
                    nc.gpsimd.dma_start(out=nt, in_=src)
                    nat_tiles[u] = nt
                else:
                    if h == 1 and (u - 1) in nat_tiles:
                        nat_tiles[u] = nat_tiles[u - 1]
                        return
                    ntb = nat_pool.tile(
                        [P, 2 * NH * SUB, D], bf16, tag="natb", name="ntb", bufs=2
                    )
                    src = x1s[b].rearrange("(u p) d -> p u d", p=P)
                    nc.gpsimd.dma_start(out=ntb, in_=src)
                    nat_tiles[2 * b] = ntb
                    nat_tiles[2 * b + 1] = ntb

            # x1 chunk loads lead the SWDGE queue so compute starts early
            load_nat(0)
            load_nat(1)

            # ---------------- constants ----------------
            ident = consts.tile([P, P], bf16, tag="ident")
            make_identity(nc, ident)
            identf = consts.tile([P, P], f32, tag="identf")
            make_identity(nc, identf)
            ones_col = consts.tile([P, 1], f32, tag="ones_col")
            nc.vector.memset(ones_col, 1.0)
            o_parts = small.tile([P, NCH * BL * NH], f32, tag="o_parts")
            xsums = small.tile([P, BL * TB * 2], f32, tag="xsums")

            # HAM warm-up: dummy matmuls keep the PE busy from t=0 so the
            # clock gate opens before the first real transposes arrive
            wscr = consts.tile([P, TH], bf16, tag="wscr")
            nc.vector.memset(wscr, 0.0)
            warm = mm_pool.tile([P, 2, TH], f32, tag="mm", name="warm")
            for w in range(28):
                nc.tensor.matmul(
                    warm[:, 0, :], lhsT=ident, rhs=wscr, start=True, stop=True
                )

            # W_h / W_a / x0 input DMAs issue early on the idle HWDGE
            # queue; the DVE cast and phase-0 compute are emitted AFTER
            # unit 0's produce so the critical x1T copies lead the queues
            wh_f32 = consts.tile([P, KC, D], f32, tag="wh_f32")
            nc.sync.dma_start(out=wh_f32, in_=wh.rearrange("(c p) n -> p c n", p=P))
            wh_mm = consts.tile([P, KC, D], dt_mm, tag="wh_mm")
            wa_sb = consts.tile([P, KC, D], f32, tag="wa")
            nc.sync.dma_start(out=wa_sb, in_=wa.rearrange("(c p) n -> p c n", p=P))
            x0_nat = small.tile([P, D], f32, tag="x0_nat")
            nc.vector.memset(x0_nat, 0.0)
            nc.sync.dma_start(out=x0_nat[:BL, :], in_=x0s)
            va_sb = consts.tile([P, T], bf16 if stt_bf16 else f32, tag="va")
            s1t_sb = small.tile([P, NCH, BL], f32, tag="s1t")

            def emit_consts_phase0():
                nc.vector.tensor_scalar(
                    out=wh_mm,
                    in0=wh_f32,
                    scalar1=WH_SCALE if mode == "fp8" else 1.0,
                    scalar2=None,
                    op0=mybir.AluOpType.mult,
                )
                va_bcast = bass.AP(
                    tensor=va.tensor, offset=va.offset, ap=[[0, P], va.ap[-1]]
                )
                nc.gpsimd.dma_start(out=va_sb, in_=va_bcast)

                # phase 0: s1T = (x0 @ W_a)^T
                x0t_sb = small.tile([P, KC, BL], f32, tag="x0t")
                for k in range(KC):
                    ps = tp_pool.tile([P, 2, TH], f32, tag="tp")
                    nc.tensor.transpose(
                        ps[:, 0, :P], x0_nat[:, k * P : (k + 1) * P], identf
                    )
                    nc.vector.tensor_copy(out=x0t_sb[:, k, :], in_=ps[:, 0, :BL])
                for n in range(NCH):
                    ps = mm_pool.tile([P, 2, TH], f32, tag="mm")
                    for k in range(KC):
                        nc.tensor.matmul(
                            ps[:, 0, :BL],
                            lhsT=wa_sb[:, k, n * P : (n + 1) * P],
                            rhs=x0t_sb[:, k, :],
                            start=(k == 0),
                            stop=(k == KC - 1),
                        )
                    nc.vector.tensor_copy(out=s1t_sb[:, n, :], in_=ps[:, 0, :BL])

            # ---------------- main loop (software-pipelined) ----------------

            def produce(u, k):
                """Unit k in 0..3 of half u: 8 transposes + 1 cast-copy."""
                b, h = units[u]
                q, pair = k // 2, k % 2
                tb = h * 2 + q
                nat = nat_tiles[u]
                hoff = (h * NH * SUB) if nat.shape[1] == 2 * NH * SUB else 0
                tp = tp_pool.tile([P, 2, TH], f32, tag="tp", name="tp")
                for dk2 in range(2):
                    dk = pair * 2 + dk2
                    for s in range(SUB):
                        uu = hoff + q * SUB + s
                        nc.tensor.matmul(
                            tp[:, dk2, s * P : (s + 1) * P],
                            lhsT=nat[:, uu, dk * P : (dk + 1) * P],
                            rhs=ident,
                            start=True,
                            stop=True,
                        )
                cidx = (b * TB + tb) * 2 + pair
                if copies_act == 5:  # alternate 3/2 -> 2.5 average on ACT
                    use_act = k < (3 if u % 2 == 0 else 2)
                else:
                    use_act = k < copies_act
                if use_act:
                    nc.scalar.activation(
                        out=xt_cur[pair][:, :, q, :],
                        in_=tp,
                        func=mybir.ActivationFunctionType.Copy,
                        accum_out=xsums[:, cidx : cidx + 1],
                    )
                else:
                    nc.vector.tensor_scalar(
                        out=xt_cur[pair][:, :, q, :],
                        in0=tp,
                        scalar1=0.0,
                        scalar2=0.0,
                        op0=mybir.AluOpType.add,
                        op1=mybir.AluOpType.add,
                        accum_out=xsums[:, cidx : cidx + 1],
                    )

            def consume(u, n):
                """n-chunk n of half u: 4 matmuls + tanh + V_a STT."""
                b, h = units[u]
                xts = xt_done  # noqa: F821 — bound by the scheduling loop
                mm = mm_pool.tile([P, 2, TH], f32, tag="mm", name="mm")
                for q in range(2):
                    if mode == "fp8":
                        for pair in range(2):
                            nc.tensor.matmul(
                                mm[:, q, :],
                                lhsT=wh_mm[
                                    :, 2 * pair : 2 * pair + 2, n * P : (n + 1) * P
                                ],
                                rhs=xts[pair][:, :, q, :],
                                start=(pair == 0),
                                stop=(pair == 1),
                                perf_mode=mybir.MatmulPerfMode.DoubleRow,
                            )
                    else:
                        for dk in range(KC):
                            nc.tensor.matmul(
                                mm[:, q, :],
                                lhsT=wh_mm[:, dk, n * P : (n + 1) * P],
                                rhs=xts[dk // 2][:, dk % 2, q, :],
                                start=(dk == 0),
                                stop=(dk == KC - 1),
                            )
                s_sb = s_pool.tile(
                    [P, NH * TH], bf16 if stt_bf16 else f32, tag="s", name="s_sb"
                )
                nc.scalar.activation(
                    out=s_sb,
                    in_=mm,
                    func=mybir.ActivationFunctionType.Tanh,
                    bias=s1t_sb[:, n, b : b + 1],
                    scale=inv_scale,
                )
                idx = (n * BL + b) * NH + h
                if stt_gps == 9:
                    # bf16 tensor_tensor at 2x, then the t-reduction fanned
                    # out to GPSIMD (DVE for one chunk)
                    gscr = scr_pool.tile(
                        [P, NH * TH], bf16, tag="gscr", name="gscr"
                    )
                    nc.vector.tensor_tensor(
                        out=gscr,
                        in0=s_sb,
                        in1=va_sb[:, h * 1024 : (h + 1) * 1024],
                        op=mybir.AluOpType.mult,
                    )
                    nc.vector.reduce_sum(
                        out=o_parts[:, idx : idx + 1],
                        in_=gscr,
                        axis=mybir.AxisListType.X,
                    )
                else:
                    scr = scr_pool.tile([P, NH * TH], f32, tag="scr", name="scr")
                    nc.vector.scalar_tensor_tensor(
                        out=scr,
                        in0=s_sb,
                        scalar=1.0,
                        in1=va_sb[:, h * 1024 : (h + 1) * 1024],
                        op0=mybir.AluOpType.mult,
                        op1=mybir.AluOpType.mult,
                        accum_out=o_parts[:, idx : idx + 1],
                    )

            # unit 0 produced ahead of the weight-cast/phase-0 emission so
            # its copies lead the DVE/ACT queues (avoids the fill-phase PE
            # starvation that re-throttles the HAM clock gate)
            xt_cur = [
                xt_pool.tile([P, 2, 2, TH], dt_mm, tag=f"xt{pair}", name=f"xt{pair}")
                for pair in range(2)
            ]
            for k in range(4):
                produce(0, k)
            hist = [xt_cur]
            # second warm-up burst: bridges the PE gap while unit 0's
            # copies drain (tp double-buffering is only 2 deep), keeping
            # the HAM clock gate open through the pipeline fill
            for w in range(14):
                nc.tensor.matmul(
                    warm[:, 1, :], lhsT=ident, rhs=wscr, start=True, stop=True
                )
            emit_consts_phase0()
            load_nat(2)
            for u in range(1, len(units)):
                load_nat(u + 2)
                xt_cur = [
                    xt_pool.tile(
                        [P, 2, 2, TH], dt_mm, tag=f"xt{pair}", name=f"xt{pair}"
                    )
                    for pair in range(2)
                ]
                for k in range(4):
                    produce(u, k)
                    if u >= skew:
                        xt_done = hist[0]
                        consume(u - skew, k)
                hist.append(xt_cur)
                if len(hist) > skew:
                    hist.pop(0)
            for uu in range(len(units) - skew, len(units)):
                xt_done = hist[0]
                for n in range(NCH):
                    consume(uu, n)
                hist.pop(0)

            # ---------------- epilogue ----------------
            o_sb = small.tile([P, NCH * BL], f32, tag="o_sb")
            nc.vector.reduce_sum(
                out=o_sb,
                in_=o_parts.rearrange("p (q t) -> p q t", t=NH),
                axis=mybir.AxisListType.X,
            )
            # transpose o -> [b, d]
            o3 = o_sb.rearrange("p (n b) -> p n b", b=BL)
            ot_ps = mm_pool.tile([P, 2, TH], f32, tag="mm")
            for n in range(NCH):
                nc.tensor.transpose(
                    ot_ps[:BL, 0, n * P : (n + 1) * P], o3[:, n, :], identf
                )
            ot_sb = small.tile([BL, D], f32, tag="ot_sb")
            nc.vector.tensor_copy(out=ot_sb, in_=ot_ps[:BL, 0, :])

            # sum(x1[b]): per-partition partials -> per-batch scalar
            xb_sb = small.tile([P, BL], f32, tag="xb_sb")
            nc.vector.reduce_sum(
                out=xb_sb,
                in_=xsums.rearrange("p (b q) -> p b q", q=TB * 2),
                axis=mybir.AxisListType.X,
            )
            sx1_ps = tp_pool.tile([P, 2, TH], f32, tag="tp")
            nc.tensor.matmul(
                sx1_ps[:BL, 0, :1], lhsT=xb_sb, rhs=ones_col, start=True, stop=True
            )
            sx1 = small.tile([BL, 1], f32, tag="sx1")
            nc.vector.tensor_copy(out=sx1, in_=sx1_ps[:BL, 0, :1])

            # softmax over D, then scale by sum(x1)
            neg_max = small.tile([BL, 1], f32, tag="neg_max")
            nc.vector.reduce_max(
                out=neg_max, in_=ot_sb, axis=mybir.AxisListType.X, negate=True
            )
            exp_sb = small.tile([BL, D], f32, tag="exp_sb")
            sum_exp = small.tile([BL, 1], f32, tag="sum_exp")
            nc.scalar.activation(
                out=exp_sb,
                in_=ot_sb,
                func=mybir.ActivationFunctionType.Exp,
                bias=neg_max,
                accum_out=sum_exp,
            )
            rec = small.tile([BL, 1], f32, tag="rec")
            nc.vector.reciprocal(out=rec, in_=sum_exp)
            scale = small.tile([BL, 1], f32, tag="scale")
            nc.vector.tensor_mul(out=scale, in0=rec, in1=sx1)
            out_sb = small.tile([BL, D], f32, tag="out_sb")
            nc.vector.tensor_scalar_mul(out=out_sb, in0=exp_sb, scalar1=scale)
            nc.sync.dma_start(out=out, in_=out_sb)

    nc.finalize()
    return nc


def make_in_maps(x0, x1, W_a, W_h, V_a):
    x0 = np.ascontiguousarray(x0, dtype=np.float32)
    x1 = np.ascontiguousarray(x1, dtype=np.float32)
    W_a = np.ascontiguousarray(W_a, dtype=np.float32)
    W_h = np.ascontiguousarray(W_h, dtype=np.float32)
    V_a = np.ascontiguousarray(V_a, dtype=np.float32)
    in_maps = []
    for c in range(NCORES):
        sl = slice(c * BL, (c + 1) * BL)
        in_maps.append(
            {
                "x0s": np.ascontiguousarray(x0[sl]),
                "x1s": np.ascontiguousarray(x1[sl]),
                "W_a": W_a,
                "W_h": W_h,
                "V_a": V_a,
            }
        )
    return in_maps


_NC_CACHE = {}


def kernel(x0, x1, W_a, W_h, V_a):
    from concourse.bass_utils import run_bass_kernel_spmd

    key = "fp8-best"
    nc = _NC_CACHE.get(key)
    if nc is None:
        nc = _NC_CACHE[key] = build_nc_v2(
            "fp8", stt_gps=0, copies_act=1, stt_bf16=True
        )
    in_maps = make_in_maps(x0, x1, W_a, W_h, V_a)
    res = run_bass_kernel_spmd(nc, in_maps, core_ids=list(range(NCORES)))
    return np.concatenate([res.results[c]["out"] for c in range(NCORES)], axis=0)


# revision 20
# speedup vs baseline: 1.1321x; 1.1321x over previous
"""Trainium2 Bass kernel for nn_AttentionLayer (B=32, T=2048, D=512).

Computation (per batch b):
    s1 = x0 @ W_a                       # (D,)
    s2 = x1[b] @ W_h                    # (T, D)
    s  = tanh(s1 + s2)                  # (T, D)
    o  = V_a @ s                        # (D,)   (contract T)
    alpha = softmax(o)                  # over D
    out[b] = alpha * sum(x1[b])

Sharding: data-parallel over batch across 8 NeuronCores (4 batches/core),
weights replicated, no cross-core communication.

v2 design notes (HW-measured 105 us vs 128 us v1 baseline; fro rel err
1.18e-2, gate 2e-2):
  v1 was elementwise-bound: ACT 76% / DVE 57% busy on [128,512] fp32 ops
  (~830 ns each incl. fixed overhead) plus per-op accumulator reads
  (ACT 378 ns). PE only 67%, first 35 us at HAM half-clock.
  v2 restructures around per-op fixed costs and engine balance:
  - t processed in 1024-wide halves: tanh/STT ops are [128,1024]
    (fixed cost amortized 2x), transpose PSUM->SBUF copies are [128,1024]
  - tanh keeps the free per-partition bias (s1T) and free scale knob
  - main matmul in fp8e4m3 with perf_mode=DoubleRow (2 k-tiles per
    instruction, ~1.8x bf16 rate, HW-verified 259 ns/MM at N=512);
    W_h pre-scaled by 64 into fp8's normal range, compensated exactly
    by tanh's scale=1/64. accum_out on the cast-copies sums the
    pre-cast fp32 PSUM values, so sum(x1) stays bf16-accurate.
  - x1 transposes as regular bf16 matmuls vs identity pipeline at
    ~67 ns/128x128 chunk (LDWEIGHTS overlaps in the 64-deep window)
  - V_a-weighted t-sums: DVE scalar_tensor_tensor, bf16 inputs
    (fp32 out+accum; the all-bf16 variant wedges TRN2, and the
    STT uop is 1x-only so bf16 only saves SBUF, not cycles)
  - software pipeline: xt production for half u emitted interleaved
    with consumption of half u-1 (skew=1; skew=2 measured worse)
  - copies: 1 on ACT / 3 on DVE per half (ACT queue stalls on the
    PE-critical path if it carries more; 2.5/1.5 measured worse)
  - DMA: batch 0 loaded as 2x2MB SWDGE cast-DMAs (prologue latency),
    batches 1-3 as 4MB each (SWDGE efficiency); W_h via HWDGE fp32 +
    on-chip cast so the first DoubleRow matmul isn't gated on the
    SWDGE queue; 28 dummy matmuls at t=0 warm the HAM clock gate and
    a second 14-matmul burst bridges the pipeline-fill gap while
    unit 0's copies drain (tp PSUM is only double-buffered); unit 0's
    produce is emitted ahead of the W_h cast + phase-0 work so the
    critical x1T copies lead the DVE/ACT queues
  - PSUM: 2x tp tiles (2 banks) + 2x mm tiles (2 banks) = 8 banks
  Engine occupancy at 105 us: DVE 64%, PE 59%, ACT 43%, SWDGE DMA 45%.
  Remaining gap to the ~60 us roofline (DMA 47 us/core, PE 52 us) is
  cross-engine dependency stalls + SBUF port contention (+20% per-op
  when fully overlapped) + HAM cold windows; deeper skew and other
  copy/engine splits measured worse (110-123 us).
  NOTE run-to-run variance: the SAME NEFF sampled 105/111/119/126 us
  across a session (HAM phase + device thermal/P0 state, trending
  slower as the device heats) — single-run A/B below ~10 us is noise.
"""

import numpy as np

B, T, D = 32, 2048, 512
NCORES = 8
BL = B // NCORES          # batches per core
P = 128
KC = D // P               # k chunks (4)
NCH = D // P              # output-d chunks (4)
TH = 512                  # t elements per 512-block
TB = T // TH              # 512-blocks (4)
NH = 2                    # 1024-halves per batch
SUB = TH // P             # 128-subtiles per 512-block (4)
WH_SCALE = 64.0           # fp8 pre-scale for W_h (into normal range)


def build_nc_v2(
    mode: str = "fp8",
    stt_gps: int = 0,
    copies_act: int = 2,
    stt_bf16: bool = False,
    skew: int = 1,
):
    """mode in {"fp8", "bf16"}.

    stt_gps: how many of the 4 per-half STT ops go to GPSIMD (rest DVE).
    NOTE: walrus rejects TensorScalarPtr on Pool — keep 0 on TRN2.
    copies_act: how many of the 4 per-half transpose-copies go to ACT
    (rest DVE).
    stt_bf16: tanh output + V_a in bf16 so the STT reads packed 16-bit
    at 2x rate (out/accum stay fp32; the known TRN2 wedge was the
    all-bf16 variant).
    """
    import concourse.bass as bass
    import concourse.tile as tile
    from concourse import bacc, mybir
    from concourse.masks import make_identity

    f32 = mybir.dt.float32
    bf16 = mybir.dt.bfloat16
    fp8 = mybir.dt.float8e4
    dt_mm = fp8 if mode == "fp8" else bf16
    inv_scale = 1.0 / WH_SCALE if mode == "fp8" else 1.0

    nc = bacc.Bacc("TRN2", target_bir_lowering=False)

    x0s = nc.dram_tensor("x0s", [BL, D], f32, kind="ExternalInput").ap()
    x1s = nc.dram_tensor("x1s", [BL, T, D], f32, kind="ExternalInput").ap()
    wa = nc.dram_tensor("W_a", [D, D], f32, kind="ExternalInput").ap()
    wh = nc.dram_tensor("W_h", [D, D], f32, kind="ExternalInput").ap()
    va = nc.dram_tensor("V_a", [1, T], f32, kind="ExternalInput").ap()
    out = nc.dram_tensor("out", [BL, D], f32, kind="ExternalOutput").ap()

    with tile.TileContext(nc) as tc:
        with (
            tc.tile_pool(name="consts", bufs=1) as consts,
            tc.tile_pool(name="nat", bufs=3) as nat_pool,
            tc.tile_pool(name="xt", bufs=4) as xt_pool,
            tc.tile_pool(name="s", bufs=3) as s_pool,
            tc.tile_pool(name="scr", bufs=2) as scr_pool,
            tc.tile_pool(name="small", bufs=1) as small,
            tc.tile_pool(name="tp_ps", bufs=2, space="PSUM") as tp_pool,
            tc.tile_pool(name="mm_ps", bufs=2, space="PSUM") as mm_pool,
        ):
            units = [(b, h) for b in range(BL) for h in range(NH)]
            nat_tiles = {}

            def load_nat(u):
                # b0 loads in 2 MB halves (prologue latency), b1.. as one
                # 4 MB transfer per batch (better SWDGE efficiency)
                if u >= len(units) or u in nat_tiles:
                    return
                b, h = units[u]
                if b == 0:
                    nt = nat_pool.tile([P, NH * SUB, D], bf16, tag="nath", name="nt")
                    src = x1s[b, h * 1024 : (h + 1) * 1024, :].rearrange(
                        "(u p) d -> p u d", p=P
                    )
                    nc.gpsimd.dma_start(out=nt, in_=src)
                    nat_tiles[u] = nt
                else:
                    if h == 1 and (u - 1) in nat_tiles:
                        nat_tiles[u] = nat_tiles[u - 1]
                        return
                    ntb = nat_pool.tile(
                        [P, 2 * NH * SUB, D], bf16, tag="natb", name="ntb", bufs=2
                    )
                    src = x1s[b].rearrange("(u p) d -> p u d", p=P)
                    nc.gpsimd.dma_start(out=ntb, in_=src)
                    nat_tiles[2 * b] = ntb
                    nat_tiles[2 * b + 1] = ntb

            # x1 chunk loads lead the SWDGE queue so compute starts early
            load_nat(0)
            load_nat(1)

            # ---------------- constants ----------------
            ident = consts.tile([P, P], bf16, tag="ident")
            make_identity(nc, ident)
            identf = consts.tile([P, P], f32, tag="identf")
            make_identity(nc, identf)
            ones_col = consts.tile([P, 1], f32, tag="ones_col")
            nc.vector.memset(ones_col, 1.0)
            o_parts = small.tile([P, NCH * BL * NH], f32, tag="o_parts")
            xsums = small.tile([P, BL * TB * 2], f32, tag="xsums")

            # HAM warm-up: dummy matmuls keep the PE busy from t=0 so the
            # clock gate opens before the first real transposes arrive
            wscr = consts.tile([P, TH], bf16, tag="wscr")
            nc.vector.memset(wscr, 0.0)
            warm = mm_pool.tile([P, 2, TH], f32, tag="mm", name="warm")
            for w in range(28):
                nc.tensor.matmul(
                    warm[:, 0, :], lhsT=ident, rhs=wscr, start=True, stop=True
                )

            # W_h / W_a / x0 input DMAs issue early on the idle HWDGE
            # queue; the DVE cast and phase-0 compute are emitted AFTER
            # unit 0's produce so the critical x1T copies lead the queues
            wh_f32 = consts.tile([P, KC, D], f32, tag="wh_f32")
            nc.sync.dma_start(out=wh_f32, in_=wh.rearrange("(c p) n -> p c n", p=P))
            wh_mm = consts.tile([P, KC, D], dt_mm, tag="wh_mm")
            wa_sb = consts.tile([P, KC, D], f32, tag="wa")
            nc.sync.dma_start(out=wa_sb, in_=wa.rearrange("(c p) n -> p c n", p=P))
            x0_nat = small.tile([P, D], f32, tag="x0_nat")
            nc.vector.memset(x0_nat, 0.0)
            nc.sync.dma_start(out=x0_nat[:BL, :], in_=x0s)
            va_sb = consts.tile([P, T], bf16 if stt_bf16 else f32, tag="va")
            s1t_sb = small.tile([P, NCH, BL], f32, tag="s1t")

            def emit_consts_phase0():
                nc.vector.tensor_scalar(
                    out=wh_mm,
                    in0=wh_f32,
                    scalar1=WH_SCALE if mode == "fp8" else 1.0,
                    scalar2=None,
                    op0=mybir.AluOpType.mult,
                )
                va_bcast = bass.AP(
                    tensor=va.tensor, offset=va.offset, ap=[[0, P], va.ap[-1]]
                )
                nc.gpsimd.dma_start(out=va_sb, in_=va_bcast)

                # phase 0: s1T = (x0 @ W_a)^T
                x0t_sb = small.tile([P, KC, BL], f32, tag="x0t")
                for k in range(KC):
                    ps = tp_pool.tile([P, 2, TH], f32, tag="tp")
                    nc.tensor.transpose(
                        ps[:, 0, :P], x0_nat[:, k * P : (k + 1) * P], identf
                    )
                    nc.vector.tensor_copy(out=x0t_sb[:, k, :], in_=ps[:, 0, :BL])
                for n in range(NCH):
                    ps = mm_pool.tile([P, 2, TH], f32, tag="mm")
                    for k in range(KC):
                        nc.tensor.matmul(
                            ps[:, 0, :BL],
                            lhsT=wa_sb[:, k, n * P : (n + 1) * P],
                            rhs=x0t_sb[:, k, :],
                            start=(k == 0),
                            stop=(k == KC - 1),
                        )
                    nc.vector.tensor_copy(out=s1t_sb[:, n, :], in_=ps[:, 0, :BL])

            # ---------------- main loop (software-pipelined) ----------------

            def produce(u, k):
                """Unit k in 0..3 of half u: 8 transposes + 1 cast-copy."""
                b, h = units[u]
                q, pair = k // 2, k % 2
                tb = h * 2 + q
                nat = nat_tiles[u]
                hoff = (h * NH * SUB) if nat.shape[1] == 2 * NH * SUB else 0
                tp = tp_pool.tile([P, 2, TH], f32, tag="tp", name="tp")
                for dk2 in range(2):
                    dk = pair * 2 + dk2
                    for s in range(SUB):
                        uu = hoff + q * SUB + s
                        nc.tensor.matmul(
                            tp[:, dk2, s * P : (s + 1) * P],
                            lhsT=nat[:, uu, dk * P : (dk + 1) * P],
                            rhs=ident,
                            start=True,
                            stop=True,
                        )
                cidx = (b * TB + tb) * 2 + pair
                if copies_act == 5:  # alternate 3/2 -> 2.5 average on ACT
                    use_act = k < (3 if u % 2 == 0 else 2)
                else:
                    use_act = k < copies_act
                if use_act:
                    nc.scalar.activation(
                        out=xt_cur[pair][:, :, q, :],
                        in_=tp,
                        func=mybir.ActivationFunctionType.Copy,
                        accum_out=xsums[:, cidx : cidx + 1],
                    )
                else:
                    nc.vector.tensor_scalar(
                        out=xt_cur[pair][:, :, q, :],
                        in0=tp,
                        scalar1=0.0,
                        scalar2=0.0,
                        op0=mybir.AluOpType.add,
                        op1=mybir.AluOpType.add,
                        accum_out=xsums[:, cidx : cidx + 1],
                    )

            def consume(u, n):
                """n-chunk n of half u: 4 matmuls + tanh + V_a STT."""
                b, h = units[u]
                xts = xt_done  # noqa: F821 — bound by the scheduling loop
                mm = mm_pool.tile([P, 2, TH], f32, tag="mm", name="mm")
                for q in range(2):
                    if mode == "fp8":
                        for pair in range(2):
                            nc.tensor.matmul(
                                mm[:, q, :],
                                lhsT=wh_mm[
                                    :, 2 * pair : 2 * pair + 2, n * P : (n + 1) * P
                                ],
                                rhs=xts[pair][:, :, q, :],
                                start=(pair == 0),
                                stop=(pair == 1),
                                perf_mode=mybir.MatmulPerfMode.DoubleRow,
                            )
                    else:
                        for dk in range(KC):
                            nc.tensor.matmul(
                                mm[:, q, :],
                                lhsT=wh_mm[:, dk, n * P : (n + 1) * P],
                                rhs=xts[dk // 2][:, dk % 2, q, :],
                                start=(dk == 0),
                                stop=(dk == KC - 1),
                            )
                s_sb = s_pool.tile(
                    [P, NH * TH], bf16 if stt_bf16 else f32, tag="s", name="s_sb"
                )
                nc.scalar.activation(
                    out=s_sb,
                    in_=mm,
                    func=mybir.ActivationFunctionType.Tanh,
                    bias=s1t_sb[:, n, b : b + 1],
                    scale=inv_scale,
                )
                idx = (n * BL + b) * NH + h
                if stt_gps == 9:
                    # bf16 tensor_tensor at 2x, then the t-reduction fanned
                    # out to GPSIMD (DVE for one chunk)
                    gscr = scr_pool.tile(
                        [P, NH * TH], bf16, tag="gscr", name="gscr"
                    )
                    nc.vector.tensor_tensor(
                        out=gscr,
                        in0=s_sb,
                        in1=va_sb[:, h * 1024 : (h + 1) * 1024],
                        op=mybir.AluOpType.mult,
                    )
                    nc.vector.reduce_sum(
                        out=o_parts[:, idx : idx + 1],
                        in_=gscr,
                        axis=mybir.AxisListType.X,
                    )
                else:
                    scr = scr_pool.tile([P, NH * TH], f32, tag="scr", name="scr")
                    nc.vector.scalar_tensor_tensor(
                        out=scr,
                        in0=s_sb,
                        scalar=1.0,
                        in1=va_sb[:, h * 1024 : (h + 1) * 1024],
                        op0=mybir.AluOpType.mult,
                        op1=mybir.AluOpType.mult,
                        accum_out=o_parts[:, idx : idx + 1],
                    )

            # unit 0 produced ahead of the weight-cast/phase-0 emission so
            # its copies lead the DVE/ACT queues (avoids the fill-phase PE
            # starvation that re-throttles the HAM clock gate)
            xt_cur = [
                xt_pool.tile([P, 2, 2, TH], dt_mm, tag=f"xt{pair}", name=f"xt{pair}")
                for pair in range(2)
            ]
            for k in range(4):
                produce(0, k)
            hist = [xt_cur]
            # second warm-up burst: bridges the PE gap while unit 0's
            # copies drain (tp double-buffering is only 2 deep), keeping
            # the HAM clock gate open through the pipeline fill
            for w in range(14):
                nc.tensor.matmul(
                    warm[:, 1, :], lhsT=ident, rhs=wscr, start=True, stop=True
                )
            emit_consts_phase0()
            load_nat(2)
            for u in range(1, len(units)):
                load_nat(u + 2)
                xt_cur = [
                    xt_pool.tile(
                        [P, 2, 2, TH], dt_mm, tag=f"xt{pair}", name=f"xt{pair}"
                    )
                    for pair in range(2)
                ]
                for k in range(4):
                    produce(u, k)
                    if u >= skew:
                        xt_done = hist[0]
                        consume(u - skew, k)
                hist.append(xt_cur)
                if len(hist) > skew:
                    hist.pop(0)
            # sum(x1) finish can overlap the drain (xsums complete already)
            xb_sb = small.tile([P, BL], f32, tag="xb_sb")
            nc.vector.reduce_sum(
                out=xb_sb,
                in_=xsums.rearrange("p (b q) -> p b q", q=TB * 2),
                axis=mybir.AxisListType.X,
            )
            sx1_ps = tp_pool.tile([P, 2, TH], f32, tag="tp")
            nc.tensor.matmul(
                sx1_ps[:BL, 0, :1], lhsT=xb_sb, rhs=ones_col, start=True, stop=True
            )
            sx1 = small.tile([BL, 1], f32, tag="sx1")
            nc.vector.tensor_copy(out=sx1, in_=sx1_ps[:BL, 0, :1])

            for uu in range(len(units) - skew, len(units)):
                xt_done = hist[0]
                for n in range(NCH):
                    consume(uu, n)
                hist.pop(0)

            # ---------------- epilogue ----------------
            o_sb = small.tile([P, NCH * BL], f32, tag="o_sb")
            nc.vector.reduce_sum(
                out=o_sb,
                in_=o_parts.rearrange("p (q t) -> p q t", t=NH),
                axis=mybir.AxisListType.X,
            )
            # transpose o -> [b, d]
            o3 = o_sb.rearrange("p (n b) -> p n b", b=BL)
            ot_ps = mm_pool.tile([P, 2, TH], f32, tag="mm")
            for n in range(NCH):
                nc.tensor.transpose(
                    ot_ps[:BL, 0, n * P : (n + 1) * P], o3[:, n, :], identf
                )
            ot_sb = small.tile([BL, D], f32, tag="ot_sb")
            nc.vector.tensor_copy(out=ot_sb, in_=ot_ps[:BL, 0, :])

            # softmax over D, then scale by sum(x1)
            neg_max = small.tile([BL, 1], f32, tag="neg_max")
            nc.vector.reduce_max(
                out=neg_max, in_=ot_sb, axis=mybir.AxisListType.X, negate=True
            )
            exp_sb = small.tile([BL, D], f32, tag="exp_sb")
            sum_exp = small.tile([BL, 1], f32, tag="sum_exp")
            nc.scalar.activation(
                out=exp_sb,
                in_=ot_sb,
                func=mybir.ActivationFunctionType.Exp,
                bias=neg_max,
                accum_out=sum_exp,
            )
            rec = small.tile([BL, 1], f32, tag="rec")
            nc.vector.reciprocal(out=rec, in_=sum_exp)
            scale = small.tile([BL, 1], f32, tag="scale")
            nc.vector.tensor_mul(out=scale, in0=rec, in1=sx1)
            out_sb = small.tile([BL, D], f32, tag="out_sb")
            nc.vector.tensor_scalar_mul(out=out_sb, in0=exp_sb, scalar1=scale)
            nc.sync.dma_start(out=out, in_=out_sb)

    nc.finalize()
    return nc


def make_in_maps(x0, x1, W_a, W_h, V_a):
    x0 = np.ascontiguousarray(x0, dtype=np.float32)
    x1 = np.ascontiguousarray(x1, dtype=np.float32)
    W_a = np.ascontiguousarray(W_a, dtype=np.float32)
    W_h = np.ascontiguousarray(W_h, dtype=np.float32)
    V_a = np.ascontiguousarray(V_a, dtype=np.float32)
    in_maps = []
    for c in range(NCORES):
        sl = slice(c * BL, (c + 1) * BL)
        in_maps.append(
            {
                "x0s": np.ascontiguousarray(x0[sl]),
                "x1s": np.ascontiguousarray(x1[sl]),
                "W_a": W_a,
                "W_h": W_h,
                "V_a": V_a,
            }
        )
    return in_maps


_NC_CACHE = {}


def kernel(x0, x1, W_a, W_h, V_a):
    from concourse.bass_utils import run_bass_kernel_spmd

    key = "fp8-best"
    nc = _NC_CACHE.get(key)
    if nc is None:
        nc = _NC_CACHE[key] = build_nc_v2(
            "fp8", stt_gps=0, copies_act=1, stt_bf16=True
        )
    in_maps = make_in_maps(x0, x1, W_a, W_h, V_a)
    res = run_bass_kernel_spmd(nc, in_maps, core_ids=list(range(NCORES)))
    return np.concatenate([res.results[c]["out"] for c in range(NCORES)], axis=0)


# revision 21
# speedup vs baseline: 1.1442x; 1.0106x over previous
"""Trainium2 Bass kernel for nn_AttentionLayer (B=32, T=2048, D=512).

Computation (per batch b):
    s1 = x0 @ W_a                       # (D,)
    s2 = x1[b] @ W_h                    # (T, D)
    s  = tanh(s1 + s2)                  # (T, D)
    o  = V_a @ s                        # (D,)   (contract T)
    alpha = softmax(o)                  # over D
    out[b] = alpha * sum(x1[b])

Sharding: data-parallel over batch across 8 NeuronCores (4 batches/core),
weights replicated, no cross-core communication.

v2 design notes (HW-measured 105 us vs 128 us v1 baseline; fro rel err
1.18e-2, gate 2e-2):
  v1 was elementwise-bound: ACT 76% / DVE 57% busy on [128,512] fp32 ops
  (~830 ns each incl. fixed overhead) plus per-op accumulator reads
  (ACT 378 ns). PE only 67%, first 35 us at HAM half-clock.
  v2 restructures around per-op fixed costs and engine balance:
  - t processed in 1024-wide halves: tanh/STT ops are [128,1024]
    (fixed cost amortized 2x), transpose PSUM->SBUF copies are [128,1024]
  - tanh keeps the free per-partition bias (s1T) and free scale knob
  - main matmul in fp8e4m3 with perf_mode=DoubleRow (2 k-tiles per
    instruction, ~1.8x bf16 rate, HW-verified 259 ns/MM at N=512);
    W_h pre-scaled by 64 into fp8's normal range, compensated exactly
    by tanh's scale=1/64. accum_out on the cast-copies sums the
    pre-cast fp32 PSUM values, so sum(x1) stays bf16-accurate.
  - x1 transposes as regular bf16 matmuls vs identity pipeline at
    ~67 ns/128x128 chunk (LDWEIGHTS overlaps in the 64-deep window)
  - V_a-weighted t-sums: DVE scalar_tensor_tensor, bf16 inputs
    (fp32 out+accum; the all-bf16 variant wedges TRN2, and the
    STT uop is 1x-only so bf16 only saves SBUF, not cycles)
  - software pipeline: xt production for half u emitted interleaved
    with consumption of half u-1 (skew=1; skew=2 measured worse)
  - copies: 1 on ACT / 3 on DVE per half (ACT queue stalls on the
    PE-critical path if it carries more; 2.5/1.5 measured worse)
  - DMA: batch 0 loaded as 2x2MB SWDGE cast-DMAs (prologue latency),
    batches 1-3 as 4MB each (SWDGE efficiency); W_h via HWDGE fp32 +
    on-chip cast so the first DoubleRow matmul isn't gated on the
    SWDGE queue; 28 dummy matmuls at t=0 warm the HAM clock gate and
    a second 14-matmul burst bridges the pipeline-fill gap while
    unit 0's copies drain (tp PSUM is only double-buffered); unit 0's
    produce is emitted ahead of the W_h cast + phase-0 work so the
    critical x1T copies lead the DVE/ACT queues
  - PSUM: 2x tp tiles (2 banks) + 2x mm tiles (2 banks) = 8 banks
  Engine occupancy at 105 us: DVE 64%, PE 59%, ACT 43%, SWDGE DMA 45%.
  Remaining gap to the ~60 us roofline (DMA 47 us/core, PE 52 us) is
  cross-engine dependency stalls + SBUF port contention (+20% per-op
  when fully overlapped) + HAM cold windows; deeper skew and other
  copy/engine splits measured worse (110-123 us).
  NOTE run-to-run variance: the SAME NEFF sampled 105/111/119/126 us
  across a session (HAM phase + device thermal/P0 state, trending
  slower as the device heats) — single-run A/B below ~10 us is noise.
"""

import numpy as np

B, T, D = 32, 2048, 512
NCORES = 8
BL = B // NCORES          # batches per core
P = 128
KC = D // P               # k chunks (4)
NCH = D // P              # output-d chunks (4)
TH = 512                  # t elements per 512-block
TB = T // TH              # 512-blocks (4)
NH = 2                    # 1024-halves per batch
SUB = TH // P             # 128-subtiles per 512-block (4)
WH_SCALE = 64.0           # fp8 pre-scale for W_h (into normal range)


def build_nc_v2(
    mode: str = "fp8",
    stt_gps: int = 0,
    copies_act: int = 2,
    stt_bf16: bool = False,
    skew: int = 1,
):
    """mode in {"fp8", "bf16"}.

    stt_gps: how many of the 4 per-half STT ops go to GPSIMD (rest DVE).
    NOTE: walrus rejects TensorScalarPtr on Pool — keep 0 on TRN2.
    copies_act: how many of the 4 per-half transpose-copies go to ACT
    (rest DVE).
    stt_bf16: tanh output + V_a in bf16 so the STT reads packed 16-bit
    at 2x rate (out/accum stay fp32; the known TRN2 wedge was the
    all-bf16 variant).
    """
    import concourse.bass as bass
    import concourse.tile as tile
    from concourse import bacc, mybir
    from concourse.masks import make_identity

    f32 = mybir.dt.float32
    bf16 = mybir.dt.bfloat16
    fp8 = mybir.dt.float8e4
    dt_mm = fp8 if mode == "fp8" else bf16
    inv_scale = 1.0 / WH_SCALE if mode == "fp8" else 1.0

    nc = bacc.Bacc("TRN2", target_bir_lowering=False)

    x0s = nc.dram_tensor("x0s", [BL, D], f32, kind="ExternalInput").ap()
    x1s = nc.dram_tensor("x1s", [BL, T, D], f32, kind="ExternalInput").ap()
    wa = nc.dram_tensor("W_a", [D, D], f32, kind="ExternalInput").ap()
    wh = nc.dram_tensor("W_h", [D, D], f32, kind="ExternalInput").ap()
    va = nc.dram_tensor("V_a", [1, T], f32, kind="ExternalInput").ap()
    out = nc.dram_tensor("out", [BL, D], f32, kind="ExternalOutput").ap()

    with tile.TileContext(nc) as tc:
        with (
            tc.tile_pool(name="consts", bufs=1) as consts,
            tc.tile_pool(name="nat", bufs=3) as nat_pool,
            tc.tile_pool(name="xt", bufs=4) as xt_pool,
            tc.tile_pool(name="s", bufs=3) as s_pool,
            tc.tile_pool(name="scr", bufs=2) as scr_pool,
            tc.tile_pool(name="small", bufs=1) as small,
            tc.tile_pool(name="tp_ps", bufs=2, space="PSUM") as tp_pool,
            tc.tile_pool(name="mm_ps", bufs=2, space="PSUM") as mm_pool,
        ):
            units = [(b, h) for b in range(BL) for h in range(NH)]
            nat_tiles = {}

            def load_nat(u):
                # b0 loads in 2 MB halves (prologue latency), b1.. as one
                # 4 MB transfer per batch (better SWDGE efficiency)
                if u >= len(units) or u in nat_tiles:
                    return
                b, h = units[u]
                if b == 0:
                    nt = nat_pool.tile([P, NH * SUB, D], bf16, tag="nath", name="nt")
                    src = x1s[b, h * 1024 : (h + 1) * 1024, :].rearrange(
                        "(u p) d -> p u d", p=P
                    )
                    nc.gpsimd.dma_start(out=nt, in_=src)
                    nat_tiles[u] = nt
                else:
                    if h == 1 and (u - 1) in nat_tiles:
                        nat_tiles[u] = nat_tiles[u - 1]
                        return
                    ntb = nat_pool.tile(
                        [P, 2 * NH * SUB, D], bf16, tag="natb", name="ntb", bufs=2
                    )
                    src = x1s[b].rearrange("(u p) d -> p u d", p=P)
                    nc.gpsimd.dma_start(out=ntb, in_=src)
                    nat_tiles[2 * b] = ntb
                    nat_tiles[2 * b + 1] = ntb

            # x1 chunk loads lead the SWDGE queue so compute starts early
            load_nat(0)
            load_nat(1)

            # ---------------- constants ----------------
            ident = consts.tile([P, P], bf16, tag="ident")
            make_identity(nc, ident)
            identf = consts.tile([P, P], f32, tag="identf")
            make_identity(nc, identf)
            ones_col = consts.tile([P, 1], f32, tag="ones_col")
            nc.vector.memset(ones_col, 1.0)
            o_parts = small.tile([P, NCH * BL * NH], f32, tag="o_parts")
            xsums = small.tile([P, BL * TB * 2], f32, tag="xsums")

            # HAM warm-up: dummy matmuls keep the PE busy from t=0 so the
            # clock gate opens before the first real transposes arrive
            wscr = consts.tile([P, TH], bf16, tag="wscr")
            nc.vector.memset(wscr, 0.0)
            # dummy tanh pulls the one-time ACT_TABLE_LOAD (~2.7 us) into
            # the idle prologue instead of stalling the first real copy on
            # the pipeline-fill critical path (tanh/copy/exp share one set)
            nc.scalar.activation(
                out=wscr[:, :1],
                in_=wscr[:, :1],
                func=mybir.ActivationFunctionType.Tanh,
            )
            warm = mm_pool.tile([P, 2, TH], f32, tag="mm", name="warm")
            for w in range(28):
                nc.tensor.matmul(
                    warm[:, 0, :], lhsT=ident, rhs=wscr, start=True, stop=True
                )

            # W_h / W_a / x0 input DMAs issue early on the idle HWDGE
            # queue; the DVE cast and phase-0 compute are emitted AFTER
            # unit 0's produce so the critical x1T copies lead the queues
            wh_f32 = consts.tile([P, KC, D], f32, tag="wh_f32")
            nc.sync.dma_start(out=wh_f32, in_=wh.rearrange("(c p) n -> p c n", p=P))
            wh_mm = consts.tile([P, KC, D], dt_mm, tag="wh_mm")
            wa_sb = consts.tile([P, KC, D], f32, tag="wa")
            nc.sync.dma_start(out=wa_sb, in_=wa.rearrange("(c p) n -> p c n", p=P))
            x0_nat = small.tile([P, D], f32, tag="x0_nat")
            nc.vector.memset(x0_nat, 0.0)
            nc.sync.dma_start(out=x0_nat[:BL, :], in_=x0s)
            va_sb = consts.tile([P, T], bf16 if stt_bf16 else f32, tag="va")
            s1t_sb = small.tile([P, NCH, BL], f32, tag="s1t")

            def emit_consts_phase0():
                nc.vector.tensor_scalar(
                    out=wh_mm,
                    in0=wh_f32,
                    scalar1=WH_SCALE if mode == "fp8" else 1.0,
                    scalar2=None,
                    op0=mybir.AluOpType.mult,
                )
                va_bcast = bass.AP(
                    tensor=va.tensor, offset=va.offset, ap=[[0, P], va.ap[-1]]
                )
                nc.gpsimd.dma_start(out=va_sb, in_=va_bcast)

                # phase 0: s1T = (x0 @ W_a)^T
                x0t_sb = small.tile([P, KC, BL], f32, tag="x0t")
                for k in range(KC):
                    ps = tp_pool.tile([P, 2, TH], f32, tag="tp")
                    nc.tensor.transpose(
                        ps[:, 0, :P], x0_nat[:, k * P : (k + 1) * P], identf
                    )
                    nc.vector.tensor_copy(out=x0t_sb[:, k, :], in_=ps[:, 0, :BL])
                for n in range(NCH):
                    ps = mm_pool.tile([P, 2, TH], f32, tag="mm")
                    for k in range(KC):
                        nc.tensor.matmul(
                            ps[:, 0, :BL],
                            lhsT=wa_sb[:, k, n * P : (n + 1) * P],
                            rhs=x0t_sb[:, k, :],
                            start=(k == 0),
                            stop=(k == KC - 1),
                        )
                    nc.vector.tensor_copy(out=s1t_sb[:, n, :], in_=ps[:, 0, :BL])

            # ---------------- main loop (software-pipelined) ----------------

            def produce(u, k):
                """Unit k in 0..3 of half u: 8 transposes + 1 cast-copy."""
                b, h = units[u]
                q, pair = k // 2, k % 2
                tb = h * 2 + q
                nat = nat_tiles[u]
                hoff = (h * NH * SUB) if nat.shape[1] == 2 * NH * SUB else 0
                tp = tp_pool.tile([P, 2, TH], f32, tag="tp", name="tp")
                for dk2 in range(2):
                    dk = pair * 2 + dk2
                    for s in range(SUB):
                        uu = hoff + q * SUB + s
                        nc.tensor.matmul(
                            tp[:, dk2, s * P : (s + 1) * P],
                            lhsT=nat[:, uu, dk * P : (dk + 1) * P],
                            rhs=ident,
                            start=True,
                            stop=True,
                        )
                cidx = (b * TB + tb) * 2 + pair
                if copies_act == 5:  # alternate 3/2 -> 2.5 average on ACT
                    use_act = k < (3 if u % 2 == 0 else 2)
                else:
                    use_act = k < copies_act
                if use_act:
                    nc.scalar.activation(
                        out=xt_cur[pair][:, :, q, :],
                        in_=tp,
                        func=mybir.ActivationFunctionType.Copy,
                        accum_out=xsums[:, cidx : cidx + 1],
                    )
                else:
                    nc.vector.tensor_scalar(
                        out=xt_cur[pair][:, :, q, :],
                        in0=tp,
                        scalar1=0.0,
                        scalar2=0.0,
                        op0=mybir.AluOpType.add,
                        op1=mybir.AluOpType.add,
                        accum_out=xsums[:, cidx : cidx + 1],
                    )

            def consume(u, n):
                """n-chunk n of half u: 4 matmuls + tanh + V_a STT."""
                b, h = units[u]
                xts = xt_done  # noqa: F821 — bound by the scheduling loop
                mm = mm_pool.tile([P, 2, TH], f32, tag="mm", name="mm")
                for q in range(2):
                    if mode == "fp8":
                        for pair in range(2):
                            nc.tensor.matmul(
                                mm[:, q, :],
                                lhsT=wh_mm[
                                    :, 2 * pair : 2 * pair + 2, n * P : (n + 1) * P
                                ],
                                rhs=xts[pair][:, :, q, :],
                                start=(pair == 0),
                                stop=(pair == 1),
                                perf_mode=mybir.MatmulPerfMode.DoubleRow,
                            )
                    else:
                        for dk in range(KC):
                            nc.tensor.matmul(
                                mm[:, q, :],
                                lhsT=wh_mm[:, dk, n * P : (n + 1) * P],
                                rhs=xts[dk // 2][:, dk % 2, q, :],
                                start=(dk == 0),
                                stop=(dk == KC - 1),
                            )
                s_sb = s_pool.tile(
                    [P, NH * TH], bf16 if stt_bf16 else f32, tag="s", name="s_sb"
                )
                nc.scalar.activation(
                    out=s_sb,
                    in_=mm,
                    func=mybir.ActivationFunctionType.Tanh,
                    bias=s1t_sb[:, n, b : b + 1],
                    scale=inv_scale,
                )
                idx = (n * BL + b) * NH + h
                if stt_gps == 9:
                    # bf16 tensor_tensor at 2x, then the t-reduction fanned
                    # out to GPSIMD (DVE for one chunk)
                    gscr = scr_pool.tile(
                        [P, NH * TH], bf16, tag="gscr", name="gscr"
                    )
                    nc.vector.tensor_tensor(
                        out=gscr,
                        in0=s_sb,
                        in1=va_sb[:, h * 1024 : (h + 1) * 1024],
                        op=mybir.AluOpType.mult,
                    )
                    nc.vector.reduce_sum(
                        out=o_parts[:, idx : idx + 1],
                        in_=gscr,
                        axis=mybir.AxisListType.X,
                    )
                else:
                    scr = scr_pool.tile([P, NH * TH], f32, tag="scr", name="scr")
                    nc.vector.scalar_tensor_tensor(
                        out=scr,
                        in0=s_sb,
                        scalar=1.0,
                        in1=va_sb[:, h * 1024 : (h + 1) * 1024],
                        op0=mybir.AluOpType.mult,
                        op1=mybir.AluOpType.mult,
                        accum_out=o_parts[:, idx : idx + 1],
                    )

            # unit 0 produced ahead of the weight-cast/phase-0 emission so
            # its copies lead the DVE/ACT queues (avoids the fill-phase PE
            # starvation that re-throttles the HAM clock gate)
            xt_cur = [
                xt_pool.tile([P, 2, 2, TH], dt_mm, tag=f"xt{pair}", name=f"xt{pair}")
                for pair in range(2)
            ]
            for k in range(4):
                produce(0, k)
            hist = [xt_cur]
            # second warm-up burst: bridges the PE gap while unit 0's
            # copies drain (tp double-buffering is only 2 deep), keeping
            # the HAM clock gate open through the pipeline fill
            for w in range(14):
                nc.tensor.matmul(
                    warm[:, 1, :], lhsT=ident, rhs=wscr, start=True, stop=True
                )
            emit_consts_phase0()
            load_nat(2)
            for u in range(1, len(units)):
                load_nat(u + 2)
                xt_cur = [
                    xt_pool.tile(
                        [P, 2, 2, TH], dt_mm, tag=f"xt{pair}", name=f"xt{pair}"
                    )
                    for pair in range(2)
                ]
                for k in range(4):
                    produce(u, k)
                    if u >= skew:
                        xt_done = hist[0]
                        consume(u - skew, k)
                hist.append(xt_cur)
                if len(hist) > skew:
                    hist.pop(0)
            # sum(x1) finish can overlap the drain (xsums complete already)
            xb_sb = small.tile([P, BL], f32, tag="xb_sb")
            nc.vector.reduce_sum(
                out=xb_sb,
                in_=xsums.rearrange("p (b q) -> p b q", q=TB * 2),
                axis=mybir.AxisListType.X,
            )
            sx1_ps = tp_pool.tile([P, 2, TH], f32, tag="tp")
            nc.tensor.matmul(
                sx1_ps[:BL, 0, :1], lhsT=xb_sb, rhs=ones_col, start=True, stop=True
            )
            sx1 = small.tile([BL, 1], f32, tag="sx1")
            nc.vector.tensor_copy(out=sx1, in_=sx1_ps[:BL, 0, :1])

            for uu in range(len(units) - skew, len(units)):
                xt_done = hist[0]
                for n in range(NCH):
                    consume(uu, n)
                hist.pop(0)

            # ---------------- epilogue ----------------
            o_sb = small.tile([P, NCH * BL], f32, tag="o_sb")
            nc.vector.reduce_sum(
                out=o_sb,
                in_=o_parts.rearrange("p (q t) -> p q t", t=NH),
                axis=mybir.AxisListType.X,
            )
            # transpose o -> [b, d]
            o3 = o_sb.rearrange("p (n b) -> p n b", b=BL)
            ot_ps = mm_pool.tile([P, 2, TH], f32, tag="mm")
            for n in range(NCH):
                nc.tensor.transpose(
                    ot_ps[:BL, 0, n * P : (n + 1) * P], o3[:, n, :], identf
                )
            ot_sb = small.tile([BL, D], f32, tag="ot_sb")
            nc.vector.tensor_copy(out=ot_sb, in_=ot_ps[:BL, 0, :])

            # softmax over D, then scale by sum(x1)
            neg_max = small.tile([BL, 1], f32, tag="neg_max")
            nc.vector.reduce_max(
                out=neg_max, in_=ot_sb, axis=mybir.AxisListType.X, negate=True
            )
            exp_sb = small.tile([BL, D], f32, tag="exp_sb")
            sum_exp = small.tile([BL, 1], f32, tag="sum_exp")
            nc.scalar.activation(
                out=exp_sb,
                in_=ot_sb,
                func=mybir.ActivationFunctionType.Exp,
                bias=neg_max,
                accum_out=sum_exp,
            )
            rec = small.tile([BL, 1], f32, tag="rec")
            nc.vector.reciprocal(out=rec, in_=sum_exp)
            scale = small.tile([BL, 1], f32, tag="scale")
            nc.vector.tensor_mul(out=scale, in0=rec, in1=sx1)
            out_sb = small.tile([BL, D], f32, tag="out_sb")
            nc.vector.tensor_scalar_mul(out=out_sb, in0=exp_sb, scalar1=scale)
            nc.sync.dma_start(out=out, in_=out_sb)

    nc.finalize()
    return nc


def make_in_maps(x0, x1, W_a, W_h, V_a):
    x0 = np.ascontiguousarray(x0, dtype=np.float32)
    x1 = np.ascontiguousarray(x1, dtype=np.float32)
    W_a = np.ascontiguousarray(W_a, dtype=np.float32)
    W_h = np.ascontiguousarray(W_h, dtype=np.float32)
    V_a = np.ascontiguousarray(V_a, dtype=np.float32)
    in_maps = []
    for c in range(NCORES):
        sl = slice(c * BL, (c + 1) * BL)
        in_maps.append(
            {
                "x0s": np.ascontiguousarray(x0[sl]),
                "x1s": np.ascontiguousarray(x1[sl]),
                "W_a": W_a,
                "W_h": W_h,
                "V_a": V_a,
            }
        )
    return in_maps


_NC_CACHE = {}


def kernel(x0, x1, W_a, W_h, V_a):
    from concourse.bass_utils import run_bass_kernel_spmd

    key = "fp8-best"
    nc = _NC_CACHE.get(key)
    if nc is None:
        nc = _NC_CACHE[key] = build_nc_v2(
            "fp8", stt_gps=0, copies_act=1, stt_bf16=True
        )
    in_maps = make_in_maps(x0, x1, W_a, W_h, V_a)
    res = run_bass_kernel_spmd(nc, in_maps, core_ids=list(range(NCORES)))
    return np.concatenate([res.results[c]["out"] for c in range(NCORES)], axis=0)
